# revision 26
# baseline (speedup 1.0000x reference)
"""Trainium2 Bass kernel for nn_CustomizeLSTMCell.

reference:
    pre = w_in_input @ s_in + w_out_input @ s_out + u_in_input @ h_in + u_out_input @ h_out
    g = sigmoid(pre)
    cell_state = g * last_c + g * g          # = g * (last_c + g)
    hidden_state = g * cell_state
    returns (cell_state, hidden_state)       # each [H, B] f32

Sharding: pure data parallel along the batch (column) axis B across 8
NeuronCores; the four [128,128] weights are replicated (pre-transposed and
packed on host so they feed the PE as lhsT directly).

The kernel is HBM-bandwidth bound (~358 GB/s per core), so all device I/O
is bf16 (tolerance is 2e-2; bf16 end-to-end lands at ~1.1e-2): 28 MiB per
core instead of 56 MiB in f32.

Device-side layout tricks (all pack/unpack happens on host):
  - the five per-tile operand tensors (s_in|s_out|h_in|h_out|last_c) are
    interleaved per 1024-column tile into one bf16 DRAM tensor, so each
    tile needs ONE 1.25 MiB load instead of 5.
  - per tile, cell/hidden outputs are written side by side in one SBUF
    tile and leave via ONE 0.5 MiB bf16 store.
  - the packed weights load issues on the sync ring BEFORE any tile load
    (on the scalar ring their completion gets stuck behind the big tile
    loads in the SDMA round-robin: a ~23 us PE warmup stall).
"""

import sys
from contextlib import ExitStack

import numpy as np
import ml_dtypes

for _p in ("/opt/trn_rl_repo", "/opt/pypackages"):
    if _p not in sys.path:
        sys.path.append(_p)

import concourse.bass as bass
import concourse.tile as tile
from concourse import bacc, mybir
from concourse import bass_utils

H = 128
S = 128
B = 131072
N_CORES = 8
B_CORE = B // N_CORES  # 16384 columns per core

N_TILE = 1024          # columns per load tile
MM_FREE = 512          # matmul free dim = one PSUM bank of f32

F32 = mybir.dt.float32
BF16 = mybir.dt.bfloat16
NP_BF16 = ml_dtypes.bfloat16

MM_INPUTS = ("s_in", "s_out", "h_in", "h_out")  # matmul operands
WEIGHTS = ("w_in_input", "w_out_input", "u_in_input", "u_out_input")
N_MM = len(MM_INPUTS)
N_OPS = N_MM + 1       # + last_c riding along in the packed tile


def tile_plan(b_core: int):
    """List of (col_offset, tile_cols). The first and final N_TILE columns
    taper (256,256,512 ... 512,256,256): a tiny first tile primes the
    whole PE->ACT->DVE->store chain quickly (so DMA completion lanes
    recycle early), and a shallow endgame minimizes the drain after the
    very last load."""
    q = N_TILE // 4
    plan = [(0, q), (q, q), (2 * q, 2 * q)]
    n_full = b_core // N_TILE - 2
    plan += [(N_TILE + i * N_TILE, N_TILE) for i in range(n_full)]
    base = (n_full + 1) * N_TILE
    plan.append((base, 2 * q))
    plan.append((base + 2 * q, q))
    plan.append((base + 3 * q, q))
    return plan


def pack_inputs(arrs, b_core: int):
    """[n][128, b_core] bf16 -> [n_rowblocks*128, n*N_TILE] tile-major: each
    tile from tile_plan() is a contiguous [a0|a1|...] block of width
    n*tile_cols; consecutive tiles fill row-blocks left to right (the
    final partial tiles share the last row-block)."""
    n_ops = len(arrs)
    n_rb = b_core // N_TILE
    out = np.empty((n_rb * H, n_ops * N_TILE), dtype=NP_BF16)
    rb, col = 0, 0
    for off, tc in tile_plan(b_core):
        blk = np.concatenate([a[:, off : off + tc] for a in arrs], axis=1)
        out[rb * H : (rb + 1) * H, col : col + n_ops * tc] = blk
        col += n_ops * tc
        if col == n_ops * N_TILE:
            rb, col = rb + 1, 0
    return out


def unpack_outputs(packed, b_core: int):
    """[n_tiles*128, 2*N_TILE] bf16 tile-major [c_t | h_t] -> (cell, hidden)
    f32, following the same row-block walk as the device stores."""
    c = np.empty((H, b_core), np.float32)
    h = np.empty((H, b_core), np.float32)
    rb, col = 0, 0
    for off, tc in tile_plan(b_core):
        blk = packed[rb * H : (rb + 1) * H, col : col + 2 * tc]
        c[:, off : off + tc] = blk[:, :tc]
        h[:, off : off + tc] = blk[:, tc:]
        col += 2 * tc
        if col == 2 * N_TILE:
            rb, col = rb + 1, 0
    return c, h


def emit_lstm_tile(ctx: ExitStack, tc: tile.TileContext, io: dict, b_core: int):
    """Per-core body.

    - loads issue on the Sync HWDGE ring, stores on the Scalar HWDGE ring
      (separate rings avoid head-of-line blocking of loads behind stores
      whose data isn't computed yet)
    - matmuls run in bf16 (full-rate PE), accumulate f32 in PSUM
    - per tile: 4 accumulating matmuls per 512-column PSUM bank group, ONE
      wide ACT sigmoid PSUM -> SBUF bf16, 3 DVE ops back to back, then one
      packed c|h store whose issue is delayed by one tile so the Scalar
      engine never stalls waiting on DVE results.
    """
    nc = tc.nc

    wpool = ctx.enter_context(tc.tile_pool(name="weights", bufs=1))
    inpool = ctx.enter_context(tc.tile_pool(name="inp", bufs=8))
    gpool = ctx.enter_context(tc.tile_pool(name="gate", bufs=4))
    tpool = ctx.enter_context(tc.tile_pool(name="tmps", bufs=2))
    opool = ctx.enter_context(tc.tile_pool(name="chout", bufs=4))
    psum = ctx.enter_context(tc.tile_pool(name="psum", bufs=4, space="PSUM"))

    # ONE packed weight load, issued on the SYNC ring BEFORE any tile load
    wt = wpool.tile([S, N_MM * H], BF16, name="w_packed")
    nc.sync.dma_start(wt[:], io["w_packed"][:])
    wtiles = [wt[:, bass.ts(k, H)] for k in range(N_MM)]

    pending_store = None  # (ch_tile, row_block, col)

    def flush_store():
        nonlocal pending_store
        if pending_store is not None:
            ch, rbo, clo = pending_store
            nc.scalar.dma_start(
                io["out_packed"][rbo * H : (rbo + 1) * H, clo : clo + ch.shape[1]],
                ch[:],
            )
            pending_store = None

    rb, col_in = 0, 0
    rb_o, col_o = 0, 0
    for off, tcols in tile_plan(b_core):
        t_in = inpool.tile([S, N_OPS * tcols], BF16, name="t_in")
        nc.sync.dma_start(
            t_in[:],
            io["in_packed"][rb * S : (rb + 1) * S, col_in : col_in + N_OPS * tcols],
        )
        col_in += N_OPS * tcols
        if col_in == N_OPS * N_TILE:
            rb, col_in = rb + 1, 0
        ops = [t_in[:, bass.ts(k, tcols)] for k in range(N_MM)]
        t_lc = t_in[:, bass.ts(N_MM, tcols)]

        # one PSUM tile spanning tcols//512 banks; 4 accumulating matmuls
        # per 512-column bank group; ONE wide sigmoid drains it to SBUF
        ps = psum.tile([H, tcols], F32, name="ps")
        for j0 in range(0, tcols, MM_FREE):
            j1 = min(j0 + MM_FREE, tcols)
            for k in range(N_MM):
                nc.tensor.matmul(
                    ps[:, j0:j1], wtiles[k], ops[k][:, j0:j1],
                    start=(k == 0), stop=(k == N_MM - 1),
                )
        g = gpool.tile([H, tcols], BF16, name="g")
        nc.scalar.activation(
            g[:], ps[:], mybir.ActivationFunctionType.Sigmoid,
        )
        flush_store()  # previous tile's c|h are ready by now

        # c = g * (last_c + g); h = g * c  -- all on DVE, back to back
        tmp = tpool.tile([H, tcols], BF16, name="tmp")
        nc.vector.tensor_add(tmp[:], g[:], t_lc[:])
        ch = opool.tile([H, 2 * tcols], BF16, name="ch")
        nc.vector.tensor_mul(ch[:, 0:tcols], g[:], tmp[:])
        nc.vector.tensor_mul(ch[:, tcols : 2 * tcols], g[:], ch[:, 0:tcols])
        pending_store = (ch, rb_o, col_o)
        col_o += 2 * tcols
        if col_o == 2 * N_TILE:
            rb_o, col_o = rb_o + 1, 0

    flush_store()


def build_model(b_core: int = B_CORE, n_cores: int = N_CORES):
    nc = bacc.Bacc(
        "TRN2",
        target_bir_lowering=False,
        debug=False,
        enable_asserts=False,
        num_devices=n_cores,
    )
    n_tiles = b_core // N_TILE
    io = {}
    io["in_packed"] = nc.dram_tensor(
        "in_packed", [n_tiles * S, N_OPS * N_TILE], BF16, kind="ExternalInput"
    ).ap()
    io["w_packed"] = nc.dram_tensor(
        "w_packed", [S, N_MM * H], BF16, kind="ExternalInput"
    ).ap()
    io["out_packed"] = nc.dram_tensor(
        "out_packed", [n_tiles * H, 2 * N_TILE], BF16, kind="ExternalOutput"
    ).ap()

    with tile.TileContext(nc) as tc, ExitStack() as ctx:
        emit_lstm_tile(ctx, tc, io, b_core)
    nc.compile()
    return nc


_model_cache: dict = {}


def _get_model():
    if "nc" not in _model_cache:
        _model_cache["nc"] = build_model()
    return _model_cache["nc"]


def make_in_maps(inputs: dict, b_core: int = B_CORE, n_cores: int = N_CORES):
    w_packed = np.ascontiguousarray(
        np.concatenate(
            [np.asarray(inputs[w]).astype(NP_BF16).T for w in WEIGHTS], axis=1
        )
    )
    big = {
        k: np.asarray(inputs[k]).astype(NP_BF16)
        for k in MM_INPUTS + ("last_c",)
    }
    in_maps = []
    for c in range(n_cores):
        sl = slice(c * b_core, (c + 1) * b_core)
        m = {
            "in_packed": pack_inputs(
                [big[k][:, sl] for k in MM_INPUTS + ("last_c",)], b_core
            ),
            "w_packed": w_packed,
        }
        in_maps.append(m)
    return in_maps


def run_spmd(inputs: dict, trace: bool = False, **kwargs):
    nc = _get_model()
    in_maps = make_in_maps(inputs)
    res = bass_utils.run_bass_kernel_spmd(
        nc, in_maps, core_ids=list(range(N_CORES)), trace=trace, **kwargs
    )
    cells, hiddens = [], []
    for c in range(N_CORES):
        cell, hidden = unpack_outputs(res.results[c]["out_packed"], B_CORE)
        cells.append(cell)
        hiddens.append(hidden)
    return (
        np.concatenate(cells, axis=1),
        np.concatenate(hiddens, axis=1),
    ), res


def kernel(**inputs):
    outs, _ = run_spmd(inputs, trace=False)
    return outs
